# revision 58
# baseline (speedup 1.0000x reference)
"""Trainium2 Bass kernel for nn_AutoGraphConstructionModule.

Pipeline (B=8, S=4096, D=1024, H=256, T=8, K=20):
  scores  = sigmoid(MLP_sc(hidden))*mask          [B,S]
  spans   = RLE of (score>=0.4 & mask), top-20 by mean span score
  reps    = mean-pooled hidden per selected span  [B,K,D]
  enhanced= MLP_en(reps)*valid                    [B,K,D]
  logits  = MLP_ty(enhanced); probs/types/escores

Sharding: data-parallel, one batch row per NeuronCore (8 cores).

Device passes (per core = one batch row):
  phase 1  scorer MLP over all 4096 tokens, bf16 matmul inputs (exact f32
           elsewhere).  hidden is shipped pre-transposed [D,S] so the
           D-contraction needs no on-device transpose.
  rescue   exact-fp32 re-score of the few hundred tokens that sit near the
           0.4 decision threshold or inside top-candidate spans, so every
           discrete decision (span RLE, top-k, ordering) matches exact-fp32
           arithmetic.
  phase 2  span mean-pool (indicator matmul) + enhancer/type MLPs in fp32.

The host only does sharding/layout, run-length encoding, and top-k index
selection on [B,S] score vectors.
"""

import sys

for _p in ("/opt/trn_rl_repo", "/root/.axon_site/_ro/trn_rl_repo"):
    if _p not in sys.path:
        sys.path.append(_p)

import numpy as np
import ml_dtypes
from contextlib import ExitStack

import concourse.bass as bass
import concourse.tile as tile
from concourse import bacc, mybir
from concourse import bass_utils

F32 = mybir.dt.float32
BF16 = mybir.dt.bfloat16
I32 = mybir.dt.int32
AF = mybir.ActivationFunctionType
OP = mybir.AluOpType

B, S, D, H, T, K = 8, 4096, 1024, 256, 8, 20
EPS = 1e-5
THRESH = np.float32(0.4)
NCORES = 8
P = 128
DC = D // P          # 8 d-chunks
NSUB = S // P        # 32 token subtiles per row
LMAX = 16            # token slots per selected span shipped to phase 2
NSELP = 384          # 20*16 padded to 3*128
SELC = NSELP // P
NRES = 256           # rescue token slots per launch
RSUB = NRES // P
BAND = np.float32(0.008)   # rescue score band around THRESH (~9x bf16 err)
NCAND = 40           # spans per row whose tokens get exact re-scoring

_cache = {}


def _newton_rstd(nc, pool, ve_ap, parts, free, tag, iters=3, pre_eps=False):
    """rstd = 1/sqrt(ve_ap [+ EPS]) on DVE only (quake seed + Newton steps).

    Keeps the ACT engine free of Sqrt so its function table never swaps
    away from the Gelu set inside the hot loop.  Rel err: ~1.7e-3 with one
    step, ~4e-6 with two, ~1.4e-7 with three.
    """
    if not pre_eps:
        ve = pool.tile([parts, free], F32, tag=tag + "ve")
        nc.vector.tensor_scalar_add(ve, ve_ap, EPS)
        ve_ap = ve[:]
    y0 = pool.tile([parts, free], I32, tag=tag + "y0")
    nc.vector.tensor_scalar(y0, ve_ap.bitcast(I32), scalar1=1, scalar2=None,
                            op0=OP.logical_shift_right)
    nc.vector.tensor_scalar(y0, y0, scalar1=0x5f3759df, scalar2=-1,
                            op0=OP.subtract, op1=OP.mult)
    y = y0[:].bitcast(F32)
    for it in range(iters):
        u = pool.tile([parts, free], F32, tag=tag + "u")
        nc.vector.tensor_mul(u, y, y)
        nc.vector.tensor_mul(u, u, ve_ap)
        nc.vector.tensor_scalar(u, u, scalar1=-0.5, scalar2=1.5,
                                op0=OP.mult, op1=OP.add)
        yn = pool.tile([parts, free], F32, tag=tag + f"y{it % 2}")
        nc.vector.tensor_mul(yn, y, u)
        y = yn
    return y


# ----------------------------------------------------------------------------
# scorer pass: per-token pre-sigmoid logits + scores.
# inputs per core: xt [D, ntok] (dtype dt), w1 [D, H] (dt), w2 [1,H] f32,
#   b2 [1,1] f32, maskt [128, nsub] f32; general adds b1,g,be [1,H] f32
# outputs: pre_out / scores_out [128, nsub]  (token t = [t%128, t//128])
# ----------------------------------------------------------------------------
def _build_scorer(general: bool, dt, nsub: int, grp: int):
    """Fast path expects w1 pre-centered on host (w1 - rowmean(w1)), so the
    matmul output already has (numerically) zero LayerNorm mean and
    var = sum(h^2)/H."""
    ntok = nsub * P
    nc = bacc.Bacc("TRN2", target_bir_lowering=False, debug=False,
                   num_devices=NCORES)
    xt = nc.dram_tensor("xt", [D, ntok], dt, kind="ExternalInput").ap()
    w1 = nc.dram_tensor("w1", [D, H], dt, kind="ExternalInput").ap()
    w2 = nc.dram_tensor("w2", [1, H], F32, kind="ExternalInput").ap()
    b2 = nc.dram_tensor("b2", [1, 1], F32, kind="ExternalInput").ap()
    maskt = nc.dram_tensor("maskt", [P, nsub], F32, kind="ExternalInput").ap()
    if general:
        b1 = nc.dram_tensor("b1", [1, H], F32, kind="ExternalInput").ap()
        g = nc.dram_tensor("g", [1, H], F32, kind="ExternalInput").ap()
        be = nc.dram_tensor("be", [1, H], F32, kind="ExternalInput").ap()
    pre_out = nc.dram_tensor("pre_out", [P, nsub], F32,
                             kind="ExternalOutput").ap()
    scores_out = nc.dram_tensor("scores_out", [P, nsub], F32,
                                kind="ExternalOutput").ap()

    with tile.TileContext(nc) as tc, ExitStack() as ctx:
        singles = ctx.enter_context(tc.tile_pool(name="singles", bufs=1))
        xpool = ctx.enter_context(tc.tile_pool(name="x",
                                               bufs=2 if grp >= 16 else 4))
        hpool = ctx.enter_context(tc.tile_pool(name="h", bufs=8, space="PSUM"))
        work = ctx.enter_context(tc.tile_pool(name="work", bufs=10))

        # w1 must land before the first matmul: first in the sync FIFO
        w1_s = singles.tile([P, DC, H], dt)
        nc.sync.dma_start(w1_s, w1.rearrange("(c p) n -> p c n", p=P))
        w2_s = singles.tile([P, H], F32)
        nc.gpsimd.dma_start(w2_s, w2.to_broadcast([P, H]))
        b2_s = singles.tile([P, 1], F32)
        nc.gpsimd.dma_start(b2_s, b2.to_broadcast([P, 1]))
        mask_s = singles.tile([P, nsub], F32)
        nc.gpsimd.dma_start(mask_s, maskt)
        if general:
            b1_s = singles.tile([P, H], F32)
            nc.gpsimd.dma_start(b1_s, b1.to_broadcast([P, H]))
            g_s = singles.tile([P, H], F32)
            nc.gpsimd.dma_start(g_s, g.to_broadcast([P, H]))
            be_s = singles.tile([P, H], F32)
            nc.gpsimd.dma_start(be_s, be.to_broadcast([P, H]))
        pre_acc = singles.tile([P, nsub], F32)

        xt_r = xt.rearrange("(c p) t -> p c t", p=P)  # [128, DC, ntok]
        for gi in range(nsub // grp):
            xg = xpool.tile([P, DC, grp * P], dt)
            half_t = (min(4, grp) if nsub > 2 else 1) * P
            for hh in range(grp * P // half_t):
                nc.sync.dma_start(
                    xg[:, :, hh * half_t:(hh + 1) * half_t],
                    xt_r[:, :, gi * grp * P + hh * half_t:
                         gi * grp * P + (hh + 1) * half_t])
            mvall = work.tile([P, grp, 2], F32, tag="mvall")
            scrg = work.tile([P, grp, H], F32, tag="scrg", bufs=2)
            hlist = []
            # small launches (rescue): one stats batch; big: batches of 4
            # (PSUM holds 8 banks, so 4 in flight + 4 recycling)
            half = grp if nsub <= 2 else min(4, grp)
            nit = 3 if dt == F32 else 1

            def stats_and_gelu(a):
                """LN rstd + gelu + w2-product for subtiles [a, a+half)."""
                hw = min(half, grp - a)
                rstd = _newton_rstd(nc, work, mvall[:, a:a + hw, 1], P,
                                    hw, "rs", iters=3 if general else nit)
                if general:
                    tneg = work.tile([P, hw], F32, tag="tneg")
                    nc.vector.tensor_tensor(tneg, mvall[:, a:a + hw, 0],
                                            rstd, OP.mult)
                    nc.vector.tensor_scalar_mul(tneg, tneg, -1.0)
                for jj in range(a, a + hw):
                    h = hlist[jj]
                    gelu_t = work.tile([P, H], F32, tag="gelu")
                    if general:
                        z = work.tile([P, H], F32, tag="z")
                        nc.vector.tensor_scalar(
                            z, h, scalar1=mvall[:, jj, 0:1],
                            scalar2=rstd[:, jj - a:jj - a + 1],
                            op0=OP.subtract, op1=OP.mult)
                        nc.vector.tensor_mul(z, z, g_s)
                        nc.vector.tensor_add(z, z, be_s)
                        nc.scalar.activation(gelu_t, z, AF.Gelu)
                    else:
                        # gelu(h * rstd) fused on ACT (h is pre-centered)
                        nc.scalar.activation(gelu_t, h, AF.Gelu,
                                             scale=rstd[:, jj - a:jj - a + 1])
                    nc.gpsimd.tensor_tensor(scrg[:, jj, :], gelu_t, w2_s,
                                            OP.mult)
                # per-batch row-sum: avoids a monolithic end-of-group DVE op
                # that would collide with the next group's rstd chain
                nc.vector.tensor_reduce(
                    pre_acc[:, gi * grp + a:gi * grp + a + hw],
                    scrg[:, a:a + hw, :], axis=mybir.AxisListType.X,
                    op=OP.add)

            if nsub <= 2:
                # interleave the subtiles' K-chunk matmuls so consecutive PE
                # ops are independent (keeps the PE clock ramped)
                hpre = [hpool.tile([P, H], F32, name=f"hp{j}", tag=f"hp{j}",
                                   bufs=1) for j in range(grp)]
                for c in range(DC):
                    for j in range(grp):
                        nc.tensor.matmul(hpre[j],
                                         lhsT=xg[:, c, j * P:(j + 1) * P],
                                         rhs=w1_s[:, c, :],
                                         start=(c == 0), stop=(c == DC - 1))
            for j in range(grp):
                if nsub <= 2:
                    h = hpre[j]
                else:
                    h = hpool.tile([P, H], F32)
                    for c in range(DC):
                        nc.tensor.matmul(h, lhsT=xg[:, c, j * P:(j + 1) * P],
                                         rhs=w1_s[:, c, :],
                                         start=(c == 0), stop=(c == DC - 1))
                if general:
                    hs = work.tile([P, H], F32, tag="hs")
                    nc.vector.tensor_add(hs, h, b1_s)
                    h = hs
                    st = work.tile([P, 6], F32, tag="st")
                    nc.vector.bn_stats(st, h)
                    nc.vector.bn_aggr(mvall[:, j, :], st)
                else:
                    # sum(h^2): alternate ACT Square-accum (Square is in
                    # every LUT set) with DVE bn_stats to balance engines
                    if j % 2 == 0:
                        sq = work.tile([P, H], F32, tag="sq")
                        nc.scalar.activation(sq, h, AF.Square,
                                             accum_out=mvall[:, j, 1:2])
                        nc.vector.tensor_scalar_mul(mvall[:, j, 1:2],
                                                    mvall[:, j, 1:2], 1.0 / H)
                    else:
                        st = work.tile([P, 6], F32, tag="st")
                        nc.vector.bn_stats(st, h)
                        nc.vector.bn_aggr(mvall[:, j, :], st)
                hlist.append(h)
                # fire each batch's LN+gelu as soon as its stats exist, so
                # PSUM banks recycle while the rest of the group matmuls
                if (j + 1) % half == 0 or j == grp - 1:
                    stats_and_gelu((j // half) * half)
            # per-group epilogue so it overlaps the next group's compute
            gsl = slice(gi * grp, (gi + 1) * grp)
            nc.vector.tensor_scalar_add(pre_acc[:, gsl], pre_acc[:, gsl],
                                        b2_s)
            nc.sync.dma_start(pre_out[:, gsl], pre_acc[:, gsl])
            sig = work.tile([P, grp], F32, tag="sig")
            nc.scalar.activation(sig, pre_acc[:, gsl], AF.Sigmoid)
            nc.vector.tensor_mul(sig, sig, mask_s[:, gsl])
            nc.sync.dma_start(scores_out[:, gsl], sig)

    nc.compile()
    return nc


# ----------------------------------------------------------------------------
# phase 2: span pooling + enhancer + type MLPs (see file docstring)
# ----------------------------------------------------------------------------
def _build_phase2(general: bool):
    nc = bacc.Bacc("TRN2", target_bir_lowering=False, debug=False,
                   num_devices=NCORES)
    xsel = nc.dram_tensor("xsel", [NSELP, D], F32, kind="ExternalInput").ap()
    ind = nc.dram_tensor("ind", [NSELP, K], F32, kind="ExternalInput").ap()
    diagv = nc.dram_tensor("diagv", [K, K], F32, kind="ExternalInput").ap()
    eye = nc.dram_tensor("eye", [K, K], F32, kind="ExternalInput").ap()
    envl = nc.dram_tensor("envl", [K, 1], F32, kind="ExternalInput").ap()
    topv = nc.dram_tensor("topv", [K, 1], F32, kind="ExternalInput").ap()
    en_w1 = nc.dram_tensor("en_w1", [D, H], F32, kind="ExternalInput").ap()
    en_w2 = nc.dram_tensor("en_w2", [H, D], F32, kind="ExternalInput").ap()
    ty_w1 = nc.dram_tensor("ty_w1", [D, H], F32, kind="ExternalInput").ap()
    ty_w2 = nc.dram_tensor("ty_w2", [H, T], F32, kind="ExternalInput").ap()
    if general:
        gens = {}
        for nm, sh in (("en_b1", [1, H]), ("en_g", [1, H]), ("en_be", [1, H]),
                       ("en_b2", [1, D]), ("ty_b1", [1, H]), ("ty_g", [1, H]),
                       ("ty_be", [1, H]), ("ty_b2", [1, T])):
            gens[nm] = nc.dram_tensor(nm, sh, F32, kind="ExternalInput").ap()
    enhanced_t = nc.dram_tensor("enhanced_t", [P, DC, K], F32,
                                kind="ExternalOutput").ap()
    logits_o = nc.dram_tensor("logits", [K, T], F32, kind="ExternalOutput").ap()
    probs_o = nc.dram_tensor("probs", [K, T], F32, kind="ExternalOutput").ap()
    escore_o = nc.dram_tensor("escore", [K, 1], F32, kind="ExternalOutput").ap()

    with tile.TileContext(nc) as tc, ExitStack() as ctx:
        singles = ctx.enter_context(tc.tile_pool(name="singles", bufs=1))
        psum = ctx.enter_context(tc.tile_pool(name="ps", bufs=1, space="PSUM"))
        work = ctx.enter_context(tc.tile_pool(name="work", bufs=2))

        # xsel/ind + enhancer weights go on the sync HWDGE ring (needed first);
        # type weights + small tensors on the gpsimd SWDGE ring so the two
        # streams overlap.
        xsel_s = singles.tile([P, SELC, D], F32)
        nc.sync.dma_start(xsel_s, xsel.rearrange("(c p) d -> p c d", p=P))
        ind_s = singles.tile([P, SELC, K], F32)
        nc.sync.dma_start(ind_s, ind.rearrange("(c p) k -> p c k", p=P))
        diagv_s = singles.tile([K, K], F32)
        nc.gpsimd.dma_start(diagv_s, diagv)
        eye_s = singles.tile([K, K], F32)
        nc.gpsimd.dma_start(eye_s, eye)
        envl_s = singles.tile([K, 1], F32)
        nc.gpsimd.dma_start(envl_s, envl)
        topv_s = singles.tile([K, 1], F32)
        nc.gpsimd.dma_start(topv_s, topv)
        en_w1_s = singles.tile([P, DC, H], F32)
        nc.sync.dma_start(en_w1_s, en_w1.rearrange("(c p) n -> p c n", p=P))
        en_w2_s = singles.tile([P, 2, DC, P], F32)
        nc.gpsimd.dma_start(
            en_w2_s, en_w2.rearrange("(hc p) (dc q) -> p hc dc q", p=P, q=P))
        ty_w1_s = singles.tile([P, DC, H], F32)
        nc.gpsimd.dma_start(ty_w1_s, ty_w1.rearrange("(c p) n -> p c n", p=P))
        ty_w2_s = singles.tile([P, 2, T], F32)
        nc.gpsimd.dma_start(ty_w2_s, ty_w2.rearrange("(hc p) t -> p hc t", p=P))
        if general:
            gen_s = {}
            for nm, w in (("en_b1", H), ("en_g", H), ("en_be", H),
                          ("ty_b1", H), ("ty_g", H), ("ty_be", H),
                          ("ty_b2", T)):
                tl = singles.tile([K, w], F32)
                nc.gpsimd.dma_start(tl, gens[nm].to_broadcast([K, w]))
                gen_s[nm] = tl
            en_b2t = singles.tile([P, DC], F32)
            nc.sync.dma_start(
                en_b2t, gens["en_b2"].rearrange("o (c p) -> (o p) c", p=P))
            valid_r = singles.tile([P, K], F32)
            nc.gpsimd.dma_start(valid_r, envl.rearrange("k o -> o k")
                                .to_broadcast([P, K]))

        def mlp_head(h_ps, gpref):
            """LN + gelu on h_ps [K, H] -> sbuf tile [K, H]."""
            st = work.tile([K, 6], F32, tag="st")
            mv = work.tile([K, 2], F32, tag="mv")
            gel = work.tile([K, H], F32, tag="gel")
            if general:
                hb = work.tile([K, H], F32, tag="hb")
                nc.vector.tensor_add(hb, h_ps, gen_s[gpref + "_b1"])
                h_ps = hb
            nc.vector.bn_stats(st, h_ps)
            nc.vector.bn_aggr(mv, st)
            rstd = _newton_rstd(nc, work, mv[:, 1:2], K, 1, "rs")
            if general:
                z = work.tile([K, H], F32, tag="z")
                nc.vector.tensor_scalar(z, h_ps, scalar1=mv[:, 0:1],
                                        scalar2=rstd, op0=OP.subtract,
                                        op1=OP.mult)
                nc.vector.tensor_mul(z, z, gen_s[gpref + "_g"])
                nc.vector.tensor_add(z, z, gen_s[gpref + "_be"])
                nc.scalar.activation(gel, z, AF.Gelu)
            else:
                tneg = work.tile([K, 1], F32, tag="tneg")
                nc.vector.tensor_scalar(tneg, mv[:, 0:1], scalar1=-1.0,
                                        scalar2=rstd, op0=OP.mult, op1=OP.mult)
                nc.scalar.activation(gel, h_ps, AF.Gelu, bias=tneg, scale=rstd)
            return gel

        # ---- reps^T [d, k] via indicator matmul (valid/len baked into ind)
        repsT_ps = psum.tile([P, DC, K], F32)
        for dci in range(DC):
            for c in range(SELC):
                nc.tensor.matmul(repsT_ps[:, dci, :],
                                 lhsT=xsel_s[:, c, dci * P:(dci + 1) * P],
                                 rhs=ind_s[:, c, :],
                                 start=(c == 0), stop=(c == SELC - 1))
        repsT = singles.tile([P, DC, K], F32)
        nc.scalar.copy(repsT, repsT_ps)

        # ---- enhancer MLP
        hen_ps = psum.tile([K, H], F32, tag="hmm")
        for c in range(DC):
            nc.tensor.matmul(hen_ps, lhsT=repsT[:, c, :], rhs=en_w1_s[:, c, :],
                             start=(c == 0), stop=(c == DC - 1))
        gel_en = mlp_head(hen_ps, "en")
        # gelu^T with valid baked in: [h, k] = gelu.T @ diag(valid)
        gelT_ps = psum.tile([P, 2, K], F32, tag="gelT")
        for hc in range(2):
            nc.tensor.matmul(gelT_ps[:, hc, :],
                             lhsT=gel_en[:, hc * P:(hc + 1) * P], rhs=diagv_s,
                             start=True, stop=True)
        gelT = work.tile([P, 2, K], F32, tag="gelTs")
        nc.scalar.copy(gelT, gelT_ps)
        enhT_ps = psum.tile([P, DC, K], F32, tag="enhT")
        for dci in range(DC):
            for hc in range(2):
                nc.tensor.matmul(enhT_ps[:, dci, :],
                                 lhsT=en_w2_s[:, hc, dci, :],
                                 rhs=gelT[:, hc, :],
                                 start=(hc == 0), stop=(hc == 1))
        enhT = singles.tile([P, DC, K], F32)
        nc.scalar.copy(enhT, enhT_ps)
        if general:
            for c in range(DC):
                nc.vector.tensor_scalar_add(enhT[:, c, :], enhT[:, c, :],
                                            en_b2t[:, c:c + 1])
                nc.vector.tensor_mul(enhT[:, c, :], enhT[:, c, :], valid_r)
        nc.sync.dma_start(enhanced_t, enhT)

        # ---- type MLP on enhanced
        hty_ps = psum.tile([K, H], F32, tag="hmm")
        for c in range(DC):
            nc.tensor.matmul(hty_ps, lhsT=enhT[:, c, :], rhs=ty_w1_s[:, c, :],
                             start=(c == 0), stop=(c == DC - 1))
        gel_ty = mlp_head(hty_ps, "ty")
        gtyT_ps = psum.tile([P, 2, K], F32, tag="gelT")
        for hc in range(2):
            nc.tensor.matmul(gtyT_ps[:, hc, :],
                             lhsT=gel_ty[:, hc * P:(hc + 1) * P], rhs=eye_s,
                             start=True, stop=True)
        gtyT = work.tile([P, 2, K], F32, tag="gtyTs")
        nc.scalar.copy(gtyT, gtyT_ps)
        log_ps = psum.tile([K, T], F32, tag="log")
        for hc in range(2):
            nc.tensor.matmul(log_ps, lhsT=gtyT[:, hc, :],
                             rhs=ty_w2_s[:, hc, :],
                             start=(hc == 0), stop=(hc == 1))
        logit = work.tile([K, T], F32, tag="logit")
        nc.scalar.copy(logit, log_ps)
        if general:
            nc.vector.tensor_add(logit, logit, gen_s["ty_b2"])
        nc.sync.dma_start(logits_o, logit)

        # softmax along T
        mx = work.tile([K, 1], F32, tag="mx")
        nc.vector.tensor_reduce(mx, logit, axis=mybir.AxisListType.X,
                                op=OP.max)
        sh = work.tile([K, T], F32, tag="sh")
        nc.vector.tensor_scalar(sh, logit, scalar1=mx, scalar2=None,
                                op0=OP.subtract)
        ex = work.tile([K, T], F32, tag="ex")
        nc.scalar.activation(ex, sh, AF.Exp)
        sm = work.tile([K, 1], F32, tag="sm")
        nc.vector.tensor_reduce(sm, ex, axis=mybir.AxisListType.X, op=OP.add)
        rs = work.tile([K, 1], F32, tag="rs")
        nc.vector.reciprocal(rs, sm)
        pr = work.tile([K, T], F32, tag="pr")
        nc.vector.tensor_scalar_mul(pr, ex, rs)
        nc.sync.dma_start(probs_o, pr)

        esc = work.tile([K, 1], F32, tag="esc")
        nc.vector.tensor_mul(esc, topv_s, envl_s)
        nc.sync.dma_start(escore_o, esc)

    nc.compile()
    return nc


def _get(key, builder):
    if key not in _cache:
        _cache[key] = builder()
    return _cache[key]


def _sigmoid64(x):
    return (1.0 / (1.0 + np.exp(-x.astype(np.float64)))).astype(np.float32)


def _rle(sc, mask_row):
    """Run-length encode spans; mirrors the reference's segment math."""
    ab = (sc >= THRESH) & (mask_row > 0)
    prev = np.zeros_like(ab)
    prev[1:] = ab[:-1]
    start = ab & ~prev
    sid = np.maximum(np.cumsum(start.astype(np.int64)) - 1, 0)
    span_len = np.zeros(S, np.float32)
    span_sum = np.zeros(S, np.float32)
    np.add.at(span_len, sid[ab], np.float32(1.0))
    np.add.at(span_sum, sid[ab], sc[ab].astype(np.float32))
    denom = np.maximum(span_len, np.float32(1.0))
    avg = span_sum / denom
    valid = (span_len > 0) & (avg >= THRESH)
    sel = np.where(valid, avg, np.float32(-1.0))
    starts_idx = np.nonzero(start)[0]
    return ab, sid, span_len, denom, sel, starts_idx


def kernel(hidden_states, attention_mask,
           sc_w1, sc_b1, sc_g, sc_be, sc_w2, sc_b2,
           en_w1, en_b1, en_g, en_be, en_w2, en_b2,
           ty_w1, ty_b1, ty_g, ty_be, ty_w2, ty_b2):
    hidden_states = np.asarray(hidden_states, dtype=np.float32)
    attention_mask = np.asarray(attention_mask)
    ws = {k: np.asarray(v, dtype=np.float32) for k, v in dict(
        sc_w1=sc_w1, sc_b1=sc_b1, sc_g=sc_g, sc_be=sc_be, sc_w2=sc_w2,
        sc_b2=sc_b2, en_w1=en_w1, en_b1=en_b1, en_g=en_g, en_be=en_be,
        en_w2=en_w2, en_b2=en_b2, ty_w1=ty_w1, ty_b1=ty_b1, ty_g=ty_g,
        ty_be=ty_be, ty_w2=ty_w2, ty_b2=ty_b2).items()}

    general = not (
        np.all(ws["sc_b1"] == 0) and np.all(ws["sc_g"] == 1)
        and np.all(ws["sc_be"] == 0) and np.all(ws["en_b1"] == 0)
        and np.all(ws["en_g"] == 1) and np.all(ws["en_be"] == 0)
        and np.all(ws["en_b2"] == 0) and np.all(ws["ty_b1"] == 0)
        and np.all(ws["ty_g"] == 1) and np.all(ws["ty_be"] == 0)
        and np.all(ws["ty_b2"] == 0))

    mask_f = attention_mask.astype(np.float32)
    cores = list(range(NCORES))

    # ---- phase 1: per-token scores ------------------------------------
    # center w1 rows (in f64) so LN's mean is zero by construction
    w1c = (ws["sc_w1"].astype(np.float64)
           - ws["sc_w1"].astype(np.float64).mean(axis=1, keepdims=True)
           ).astype(np.float32)
    if general:
        nc1 = _get(("scorer", "f32g", NSUB), lambda: _build_scorer(
            True, F32, NSUB, 4))
        xt_cast = lambda a: np.ascontiguousarray(a.T)
        w1_cast = ws["sc_w1"]
    else:
        nc1 = _get(("scorer", "bf16", NSUB), lambda: _build_scorer(
            False, BF16, NSUB, 16))
        xt_cast = lambda a: np.ascontiguousarray(a.T).astype(
            ml_dtypes.bfloat16)
        w1_cast = w1c.astype(ml_dtypes.bfloat16)

    in_maps1 = []
    for r in range(B):
        m = {
            "xt": xt_cast(hidden_states[r]),
            "w1": w1_cast,
            "w2": np.ascontiguousarray(ws["sc_w2"].reshape(1, H)),
            "b2": ws["sc_b2"].reshape(1, 1),
            "maskt": np.ascontiguousarray(mask_f[r].reshape(NSUB, P).T),
        }
        if general:
            m["b1"] = ws["sc_b1"].reshape(1, H)
            m["g"] = ws["sc_g"].reshape(1, H)
            m["be"] = ws["sc_be"].reshape(1, H)
        in_maps1.append(m)
    r1 = bass_utils.run_bass_kernel_spmd(nc1, in_maps1, core_ids=cores)
    pre = np.stack([r1.results[r]["pre_out"].T.reshape(S) for r in range(B)])
    scores_dev = np.stack(
        [r1.results[r]["scores_out"].T.reshape(S) for r in range(B)])

    # ---- rescue: exact fp32 re-score of decision-critical tokens -------
    if not general:
        s_b = _sigmoid64(pre) * mask_f
        rescue_idx = []
        for r in range(B):
            ab, sid, span_len, denom, sel, starts_idx = _rle(s_b[r], mask_f[r])
            keep = set()
            band = np.nonzero(
                (np.abs(s_b[r] - THRESH) < BAND) & (mask_f[r] > 0))[0]
            nspans = len(starts_idx)

            def add_span(j):
                if 0 <= j < nspans:
                    st0 = int(starts_idx[j])
                    keep.update(range(st0, st0 + int(span_len[j])))

            for t in band:
                keep.add(int(t))
                for tt in (t - 1, t + 1):
                    if 0 <= tt < S and ab[tt]:
                        add_span(int(sid[tt]))
            for j in np.argsort(-sel, kind="stable")[:NCAND]:
                add_span(int(j))
            rescue_idx.append(np.array(sorted(keep), dtype=np.int64))

        ncre = _get(("scorer", "f32", RSUB), lambda: _build_scorer(
            False, F32, RSUB, RSUB))
        nchunks = max(1, max((len(i) + NRES - 1) // NRES for i in rescue_idx))
        for ch in range(nchunks):
            in_mapsr = []
            for r in range(B):
                idx = rescue_idx[r][ch * NRES:(ch + 1) * NRES]
                xr = np.zeros((NRES, D), np.float32)
                xr[:len(idx)] = hidden_states[r, idx]
                in_mapsr.append({
                    "xt": np.ascontiguousarray(xr.T),
                    "w1": w1c,
                    "w2": np.ascontiguousarray(ws["sc_w2"].reshape(1, H)),
                    "b2": ws["sc_b2"].reshape(1, 1),
                    "maskt": np.ones((P, RSUB), np.float32),
                })
            rr = bass_utils.run_bass_kernel_spmd(ncre, in_mapsr,
                                                 core_ids=cores)
            for r in range(B):
                idx = rescue_idx[r][ch * NRES:(ch + 1) * NRES]
                if len(idx) == 0:
                    continue
                pre_r = rr.results[r]["pre_out"].T.reshape(NRES)[:len(idx)]
                s_r = rr.results[r]["scores_out"].T.reshape(NRES)[:len(idx)]
                pre[r, idx] = pre_r
                scores_dev[r, idx] = s_r * mask_f[r, idx]

    # ---- host: RLE + top-k selection (index bookkeeping) ---------------
    s_sel = _sigmoid64(pre) * mask_f
    in_maps2 = []
    sel_meta = []
    for r in range(B):
        ab, sid, span_len, denom, sel, starts_idx = _rle(s_sel[r], mask_f[r])
        order = np.argsort(-sel, kind="stable")[:K]
        top_vals = sel[order]
        valid_k = (top_vals >= THRESH).astype(np.float32)
        lens_k = denom[order]
        nspans = len(starts_idx)

        xsel = np.zeros((NSELP, D), np.float32)
        ind = np.zeros((NSELP, K), np.float32)
        for k in range(K):
            j = int(order[k])
            if valid_k[k] == 0 or j >= nspans:
                continue
            st_j = int(starts_idx[j])
            ln = int(span_len[j])
            row0 = k * LMAX
            scale = np.float32(valid_k[k] / lens_k[k])
            if ln <= LMAX:
                xsel[row0:row0 + ln] = hidden_states[r, st_j:st_j + ln]
                ind[row0:row0 + ln, k] = scale
            else:
                nfit = LMAX - 1
                xsel[row0:row0 + nfit] = hidden_states[r, st_j:st_j + nfit]
                xsel[row0 + nfit] = np.add.reduce(
                    hidden_states[r, st_j + nfit:st_j + ln], axis=0,
                    dtype=np.float32)
                ind[row0:row0 + LMAX, k] = scale
        m = {
            "xsel": xsel,
            "ind": ind,
            "diagv": np.diag(valid_k).astype(np.float32),
            "eye": np.eye(K, dtype=np.float32),
            "envl": valid_k.reshape(K, 1),
            "topv": top_vals.reshape(K, 1).astype(np.float32),
            "en_w1": ws["en_w1"], "en_w2": ws["en_w2"],
            "ty_w1": ws["ty_w1"], "ty_w2": ws["ty_w2"],
        }
        if general:
            m.update({
                "en_b1": ws["en_b1"].reshape(1, H),
                "en_g": ws["en_g"].reshape(1, H),
                "en_be": ws["en_be"].reshape(1, H),
                "en_b2": ws["en_b2"].reshape(1, D),
                "ty_b1": ws["ty_b1"].reshape(1, H),
                "ty_g": ws["ty_g"].reshape(1, H),
                "ty_be": ws["ty_be"].reshape(1, H),
                "ty_b2": ws["ty_b2"].reshape(1, T),
            })
        in_maps2.append(m)
        sel_meta.append((top_vals, valid_k))

    nc2 = _get(("phase2", general), lambda: _build_phase2(general))
    r2 = bass_utils.run_bass_kernel_spmd(nc2, in_maps2, core_ids=cores)

    enhanced = np.stack([
        r2.results[r]["enhanced_t"].transpose(2, 1, 0).reshape(K, D)
        for r in range(B)])
    type_logits = np.stack([r2.results[r]["logits"] for r in range(B)])
    type_probs = np.stack([r2.results[r]["probs"] for r in range(B)])
    entity_scores = np.stack(
        [r2.results[r]["escore"].reshape(K) for r in range(B)])
    entity_types = np.argmax(type_logits, axis=-1).astype(np.int32)
    valid_k_out = np.stack([sel_meta[r][1] for r in range(B)])

    return (scores_dev.astype(np.float32), enhanced, type_logits,
            entity_scores, type_probs, entity_types, valid_k_out)


# revision 59
# speedup vs baseline: 1.0153x; 1.0153x over previous
"""Trainium2 Bass kernel for nn_AutoGraphConstructionModule.

Pipeline (B=8, S=4096, D=1024, H=256, T=8, K=20):
  scores  = sigmoid(MLP_sc(hidden))*mask          [B,S]
  spans   = RLE of (score>=0.4 & mask), top-20 by mean span score
  reps    = mean-pooled hidden per selected span  [B,K,D]
  enhanced= MLP_en(reps)*valid                    [B,K,D]
  logits  = MLP_ty(enhanced); probs/types/escores

Sharding: data-parallel, one batch row per NeuronCore (8 cores).

Device passes (per core = one batch row):
  phase 1  scorer MLP over all 4096 tokens, bf16 matmul inputs (exact f32
           elsewhere).  hidden is shipped pre-transposed [D,S] so the
           D-contraction needs no on-device transpose.
  rescue   exact-fp32 re-score of the few hundred tokens that sit near the
           0.4 decision threshold or inside top-candidate spans, so every
           discrete decision (span RLE, top-k, ordering) matches exact-fp32
           arithmetic.
  phase 2  span mean-pool (indicator matmul) + enhancer/type MLPs in fp32.

The host only does sharding/layout, run-length encoding, and top-k index
selection on [B,S] score vectors.
"""

import sys

for _p in ("/opt/trn_rl_repo", "/root/.axon_site/_ro/trn_rl_repo"):
    if _p not in sys.path:
        sys.path.append(_p)

import numpy as np
import ml_dtypes
from contextlib import ExitStack

import concourse.bass as bass
import concourse.tile as tile
from concourse import bacc, mybir
from concourse import bass_utils

F32 = mybir.dt.float32
BF16 = mybir.dt.bfloat16
I32 = mybir.dt.int32
AF = mybir.ActivationFunctionType
OP = mybir.AluOpType

B, S, D, H, T, K = 8, 4096, 1024, 256, 8, 20
EPS = 1e-5
THRESH = np.float32(0.4)
NCORES = 8
P = 128
DC = D // P          # 8 d-chunks
NSUB = S // P        # 32 token subtiles per row
LMAX = 16            # token slots per selected span shipped to phase 2
NSELP = 384          # 20*16 padded to 3*128
SELC = NSELP // P
NRES = 256           # rescue token slots per launch
RSUB = NRES // P
BAND = np.float32(0.008)   # rescue score band around THRESH (~9x bf16 err)
NCAND = 40           # spans per row whose tokens get exact re-scoring

_cache = {}


def _newton_rstd(nc, pool, ve_ap, parts, free, tag, iters=3, pre_eps=False):
    """rstd = 1/sqrt(ve_ap [+ EPS]) on DVE only (quake seed + Newton steps).

    Keeps the ACT engine free of Sqrt so its function table never swaps
    away from the Gelu set inside the hot loop.  Rel err: ~1.7e-3 with one
    step, ~4e-6 with two, ~1.4e-7 with three.
    """
    if not pre_eps:
        ve = pool.tile([parts, free], F32, tag=tag + "ve")
        nc.vector.tensor_scalar_add(ve, ve_ap, EPS)
        ve_ap = ve[:]
    y0 = pool.tile([parts, free], I32, tag=tag + "y0")
    nc.vector.tensor_scalar(y0, ve_ap.bitcast(I32), scalar1=1, scalar2=None,
                            op0=OP.logical_shift_right)
    nc.vector.tensor_scalar(y0, y0, scalar1=0x5f3759df, scalar2=-1,
                            op0=OP.subtract, op1=OP.mult)
    y = y0[:].bitcast(F32)
    for it in range(iters):
        u = pool.tile([parts, free], F32, tag=tag + "u")
        nc.vector.tensor_mul(u, y, y)
        nc.vector.tensor_mul(u, u, ve_ap)
        nc.vector.tensor_scalar(u, u, scalar1=-0.5, scalar2=1.5,
                                op0=OP.mult, op1=OP.add)
        yn = pool.tile([parts, free], F32, tag=tag + f"y{it % 2}")
        nc.vector.tensor_mul(yn, y, u)
        y = yn
    return y


# ----------------------------------------------------------------------------
# scorer pass: per-token pre-sigmoid logits + scores.
# inputs per core: xt [D, ntok] (dtype dt), w1 [D, H] (dt), w2 [1,H] f32,
#   b2 [1,1] f32, maskt [128, nsub] f32; general adds b1,g,be [1,H] f32
# outputs: pre_out / scores_out [128, nsub]  (token t = [t%128, t//128])
# ----------------------------------------------------------------------------
def _build_scorer(general: bool, dt, nsub: int, grp: int):
    """Fast path expects w1 pre-centered on host (w1 - rowmean(w1)), so the
    matmul output already has (numerically) zero LayerNorm mean and
    var = sum(h^2)/H."""
    ntok = nsub * P
    nc = bacc.Bacc("TRN2", target_bir_lowering=False, debug=False,
                   num_devices=NCORES)
    xt = nc.dram_tensor("xt", [D, ntok], dt, kind="ExternalInput").ap()
    w1 = nc.dram_tensor("w1", [D, H], dt, kind="ExternalInput").ap()
    w2 = nc.dram_tensor("w2", [1, H], F32, kind="ExternalInput").ap()
    b2 = nc.dram_tensor("b2", [1, 1], F32, kind="ExternalInput").ap()
    maskt = nc.dram_tensor("maskt", [P, nsub], F32, kind="ExternalInput").ap()
    if general:
        b1 = nc.dram_tensor("b1", [1, H], F32, kind="ExternalInput").ap()
        g = nc.dram_tensor("g", [1, H], F32, kind="ExternalInput").ap()
        be = nc.dram_tensor("be", [1, H], F32, kind="ExternalInput").ap()
    pre_out = nc.dram_tensor("pre_out", [P, nsub], F32,
                             kind="ExternalOutput").ap()
    scores_out = nc.dram_tensor("scores_out", [P, nsub], F32,
                                kind="ExternalOutput").ap()

    with tile.TileContext(nc) as tc, ExitStack() as ctx:
        singles = ctx.enter_context(tc.tile_pool(name="singles", bufs=1))
        xpool = ctx.enter_context(tc.tile_pool(name="x",
                                               bufs=2 if grp >= 16 else 4))
        hpool = ctx.enter_context(tc.tile_pool(name="h", bufs=8, space="PSUM"))
        work = ctx.enter_context(tc.tile_pool(name="work", bufs=10))

        # w1 must land before the first matmul: first in the sync FIFO
        w1_s = singles.tile([P, DC, H], dt)
        nc.sync.dma_start(w1_s, w1.rearrange("(c p) n -> p c n", p=P))
        w2_s = singles.tile([P, H], F32)
        nc.gpsimd.dma_start(w2_s, w2.to_broadcast([P, H]))
        b2_s = singles.tile([P, 1], F32)
        nc.gpsimd.dma_start(b2_s, b2.to_broadcast([P, 1]))
        mask_s = singles.tile([P, nsub], F32)
        nc.gpsimd.dma_start(mask_s, maskt)
        if general:
            b1_s = singles.tile([P, H], F32)
            nc.gpsimd.dma_start(b1_s, b1.to_broadcast([P, H]))
            g_s = singles.tile([P, H], F32)
            nc.gpsimd.dma_start(g_s, g.to_broadcast([P, H]))
            be_s = singles.tile([P, H], F32)
            nc.gpsimd.dma_start(be_s, be.to_broadcast([P, H]))
        pre_acc = singles.tile([P, nsub], F32)

        xt_r = xt.rearrange("(c p) t -> p c t", p=P)  # [128, DC, ntok]
        for gi in range(nsub // grp):
            xg = xpool.tile([P, DC, grp * P], dt)
            half_t = (min(4, grp) if nsub > 2 else 1) * P
            for hh in range(grp * P // half_t):
                nc.sync.dma_start(
                    xg[:, :, hh * half_t:(hh + 1) * half_t],
                    xt_r[:, :, gi * grp * P + hh * half_t:
                         gi * grp * P + (hh + 1) * half_t])
            mvall = work.tile([P, grp, 2], F32, tag="mvall")
            scrg = work.tile([P, grp, H], F32, tag="scrg", bufs=2)
            hlist = []
            # small launches (rescue): one stats batch; big: batches of 4
            # (PSUM holds 8 banks, so 4 in flight + 4 recycling)
            half = grp if nsub <= 2 else min(4, grp)
            nit = 3 if dt == F32 else 1

            def stats_and_gelu(a):
                """LN rstd + gelu + w2-product for subtiles [a, a+half)."""
                hw = min(half, grp - a)
                rstd = _newton_rstd(nc, work, mvall[:, a:a + hw, 1], P,
                                    hw, "rs", iters=3 if general else nit)
                if general:
                    tneg = work.tile([P, hw], F32, tag="tneg")
                    nc.vector.tensor_tensor(tneg, mvall[:, a:a + hw, 0],
                                            rstd, OP.mult)
                    nc.vector.tensor_scalar_mul(tneg, tneg, -1.0)
                for jj in range(a, a + hw):
                    h = hlist[jj]
                    gelu_t = work.tile([P, H], F32, tag="gelu")
                    if general:
                        z = work.tile([P, H], F32, tag="z")
                        nc.vector.tensor_scalar(
                            z, h, scalar1=mvall[:, jj, 0:1],
                            scalar2=rstd[:, jj - a:jj - a + 1],
                            op0=OP.subtract, op1=OP.mult)
                        nc.vector.tensor_mul(z, z, g_s)
                        nc.vector.tensor_add(z, z, be_s)
                        nc.scalar.activation(gelu_t, z, AF.Gelu)
                    else:
                        # gelu(h * rstd) fused on ACT (h is pre-centered)
                        nc.scalar.activation(gelu_t, h, AF.Gelu,
                                             scale=rstd[:, jj - a:jj - a + 1])
                    nc.gpsimd.tensor_tensor(scrg[:, jj, :], gelu_t, w2_s,
                                            OP.mult)
                # per-batch row-sum: avoids a monolithic end-of-group DVE op
                # that would collide with the next group's rstd chain
                nc.vector.tensor_reduce(
                    pre_acc[:, gi * grp + a:gi * grp + a + hw],
                    scrg[:, a:a + hw, :], axis=mybir.AxisListType.X,
                    op=OP.add)

            if nsub <= 2:
                # interleave the subtiles' K-chunk matmuls so consecutive PE
                # ops are independent (keeps the PE clock ramped)
                hpre = [hpool.tile([P, H], F32, name=f"hp{j}", tag=f"hp{j}",
                                   bufs=1) for j in range(grp)]
                for c in range(DC):
                    for j in range(grp):
                        nc.tensor.matmul(hpre[j],
                                         lhsT=xg[:, c, j * P:(j + 1) * P],
                                         rhs=w1_s[:, c, :],
                                         start=(c == 0), stop=(c == DC - 1))
            for j in range(grp):
                if nsub <= 2:
                    h = hpre[j]
                else:
                    h = hpool.tile([P, H], F32)
                    for c in range(DC):
                        nc.tensor.matmul(h, lhsT=xg[:, c, j * P:(j + 1) * P],
                                         rhs=w1_s[:, c, :],
                                         start=(c == 0), stop=(c == DC - 1))
                if general:
                    hs = work.tile([P, H], F32, tag="hs")
                    nc.vector.tensor_add(hs, h, b1_s)
                    h = hs
                    st = work.tile([P, 6], F32, tag="st")
                    nc.vector.bn_stats(st, h)
                    nc.vector.bn_aggr(mvall[:, j, :], st)
                else:
                    # sum(h^2): alternate ACT Square-accum (Square is in
                    # every LUT set) with DVE bn_stats to balance engines
                    if j % 2 == 0:
                        sq = work.tile([P, H], F32, tag="sq")
                        nc.scalar.activation(sq, h, AF.Square,
                                             accum_out=mvall[:, j, 1:2])
                        nc.vector.tensor_scalar_mul(mvall[:, j, 1:2],
                                                    mvall[:, j, 1:2], 1.0 / H)
                    else:
                        st = work.tile([P, 6], F32, tag="st")
                        nc.vector.bn_stats(st, h)
                        nc.vector.bn_aggr(mvall[:, j, :], st)
                hlist.append(h)
                # fire each batch's LN+gelu as soon as its stats exist, so
                # PSUM banks recycle while the rest of the group matmuls
                if (j + 1) % half == 0 or j == grp - 1:
                    stats_and_gelu((j // half) * half)
        nc.vector.tensor_scalar_add(pre_acc, pre_acc, b2_s)
        # pre_out first: it must not queue behind the sigmoid-dependent
        # scores_out on the sync FIFO
        nc.sync.dma_start(pre_out, pre_acc)
        sig = singles.tile([P, nsub], F32)
        nc.scalar.activation(sig, pre_acc, AF.Sigmoid)
        nc.vector.tensor_mul(sig, sig, mask_s)
        nc.sync.dma_start(scores_out, sig)

    nc.compile()
    return nc


# ----------------------------------------------------------------------------
# phase 2: span pooling + enhancer + type MLPs (see file docstring)
# ----------------------------------------------------------------------------
def _build_phase2(general: bool):
    nc = bacc.Bacc("TRN2", target_bir_lowering=False, debug=False,
                   num_devices=NCORES)
    xsel = nc.dram_tensor("xsel", [NSELP, D], F32, kind="ExternalInput").ap()
    ind = nc.dram_tensor("ind", [NSELP, K], F32, kind="ExternalInput").ap()
    diagv = nc.dram_tensor("diagv", [K, K], F32, kind="ExternalInput").ap()
    eye = nc.dram_tensor("eye", [K, K], F32, kind="ExternalInput").ap()
    envl = nc.dram_tensor("envl", [K, 1], F32, kind="ExternalInput").ap()
    topv = nc.dram_tensor("topv", [K, 1], F32, kind="ExternalInput").ap()
    en_w1 = nc.dram_tensor("en_w1", [D, H], F32, kind="ExternalInput").ap()
    en_w2 = nc.dram_tensor("en_w2", [H, D], F32, kind="ExternalInput").ap()
    ty_w1 = nc.dram_tensor("ty_w1", [D, H], F32, kind="ExternalInput").ap()
    ty_w2 = nc.dram_tensor("ty_w2", [H, T], F32, kind="ExternalInput").ap()
    if general:
        gens = {}
        for nm, sh in (("en_b1", [1, H]), ("en_g", [1, H]), ("en_be", [1, H]),
                       ("en_b2", [1, D]), ("ty_b1", [1, H]), ("ty_g", [1, H]),
                       ("ty_be", [1, H]), ("ty_b2", [1, T])):
            gens[nm] = nc.dram_tensor(nm, sh, F32, kind="ExternalInput").ap()
    enhanced_t = nc.dram_tensor("enhanced_t", [P, DC, K], F32,
                                kind="ExternalOutput").ap()
    logits_o = nc.dram_tensor("logits", [K, T], F32, kind="ExternalOutput").ap()
    probs_o = nc.dram_tensor("probs", [K, T], F32, kind="ExternalOutput").ap()
    escore_o = nc.dram_tensor("escore", [K, 1], F32, kind="ExternalOutput").ap()

    with tile.TileContext(nc) as tc, ExitStack() as ctx:
        singles = ctx.enter_context(tc.tile_pool(name="singles", bufs=1))
        psum = ctx.enter_context(tc.tile_pool(name="ps", bufs=1, space="PSUM"))
        work = ctx.enter_context(tc.tile_pool(name="work", bufs=2))

        # xsel/ind + enhancer weights go on the sync HWDGE ring (needed first);
        # type weights + small tensors on the gpsimd SWDGE ring so the two
        # streams overlap.
        xsel_s = singles.tile([P, SELC, D], F32)
        nc.sync.dma_start(xsel_s, xsel.rearrange("(c p) d -> p c d", p=P))
        ind_s = singles.tile([P, SELC, K], F32)
        nc.sync.dma_start(ind_s, ind.rearrange("(c p) k -> p c k", p=P))
        diagv_s = singles.tile([K, K], F32)
        nc.gpsimd.dma_start(diagv_s, diagv)
        eye_s = singles.tile([K, K], F32)
        nc.gpsimd.dma_start(eye_s, eye)
        envl_s = singles.tile([K, 1], F32)
        nc.gpsimd.dma_start(envl_s, envl)
        topv_s = singles.tile([K, 1], F32)
        nc.gpsimd.dma_start(topv_s, topv)
        en_w1_s = singles.tile([P, DC, H], F32)
        nc.sync.dma_start(en_w1_s, en_w1.rearrange("(c p) n -> p c n", p=P))
        en_w2_s = singles.tile([P, 2, DC, P], F32)
        nc.gpsimd.dma_start(
            en_w2_s, en_w2.rearrange("(hc p) (dc q) -> p hc dc q", p=P, q=P))
        ty_w1_s = singles.tile([P, DC, H], F32)
        nc.gpsimd.dma_start(ty_w1_s, ty_w1.rearrange("(c p) n -> p c n", p=P))
        ty_w2_s = singles.tile([P, 2, T], F32)
        nc.gpsimd.dma_start(ty_w2_s, ty_w2.rearrange("(hc p) t -> p hc t", p=P))
        if general:
            gen_s = {}
            for nm, w in (("en_b1", H), ("en_g", H), ("en_be", H),
                          ("ty_b1", H), ("ty_g", H), ("ty_be", H),
                          ("ty_b2", T)):
                tl = singles.tile([K, w], F32)
                nc.gpsimd.dma_start(tl, gens[nm].to_broadcast([K, w]))
                gen_s[nm] = tl
            en_b2t = singles.tile([P, DC], F32)
            nc.sync.dma_start(
                en_b2t, gens["en_b2"].rearrange("o (c p) -> (o p) c", p=P))
            valid_r = singles.tile([P, K], F32)
            nc.gpsimd.dma_start(valid_r, envl.rearrange("k o -> o k")
                                .to_broadcast([P, K]))

        def mlp_head(h_ps, gpref):
            """LN + gelu on h_ps [K, H] -> sbuf tile [K, H]."""
            st = work.tile([K, 6], F32, tag="st")
            mv = work.tile([K, 2], F32, tag="mv")
            gel = work.tile([K, H], F32, tag="gel")
            if general:
                hb = work.tile([K, H], F32, tag="hb")
                nc.vector.tensor_add(hb, h_ps, gen_s[gpref + "_b1"])
                h_ps = hb
            nc.vector.bn_stats(st, h_ps)
            nc.vector.bn_aggr(mv, st)
            rstd = _newton_rstd(nc, work, mv[:, 1:2], K, 1, "rs")
            if general:
                z = work.tile([K, H], F32, tag="z")
                nc.vector.tensor_scalar(z, h_ps, scalar1=mv[:, 0:1],
                                        scalar2=rstd, op0=OP.subtract,
                                        op1=OP.mult)
                nc.vector.tensor_mul(z, z, gen_s[gpref + "_g"])
                nc.vector.tensor_add(z, z, gen_s[gpref + "_be"])
                nc.scalar.activation(gel, z, AF.Gelu)
            else:
                tneg = work.tile([K, 1], F32, tag="tneg")
                nc.vector.tensor_scalar(tneg, mv[:, 0:1], scalar1=-1.0,
                                        scalar2=rstd, op0=OP.mult, op1=OP.mult)
                nc.scalar.activation(gel, h_ps, AF.Gelu, bias=tneg, scale=rstd)
            return gel

        # ---- reps^T [d, k] via indicator matmul (valid/len baked into ind)
        repsT_ps = psum.tile([P, DC, K], F32)
        for dci in range(DC):
            for c in range(SELC):
                nc.tensor.matmul(repsT_ps[:, dci, :],
                                 lhsT=xsel_s[:, c, dci * P:(dci + 1) * P],
                                 rhs=ind_s[:, c, :],
                                 start=(c == 0), stop=(c == SELC - 1))
        repsT = singles.tile([P, DC, K], F32)
        nc.scalar.copy(repsT, repsT_ps)

        # ---- enhancer MLP
        hen_ps = psum.tile([K, H], F32, tag="hmm")
        for c in range(DC):
            nc.tensor.matmul(hen_ps, lhsT=repsT[:, c, :], rhs=en_w1_s[:, c, :],
                             start=(c == 0), stop=(c == DC - 1))
        gel_en = mlp_head(hen_ps, "en")
        # gelu^T with valid baked in: [h, k] = gelu.T @ diag(valid)
        gelT_ps = psum.tile([P, 2, K], F32, tag="gelT")
        for hc in range(2):
            nc.tensor.matmul(gelT_ps[:, hc, :],
                             lhsT=gel_en[:, hc * P:(hc + 1) * P], rhs=diagv_s,
                             start=True, stop=True)
        gelT = work.tile([P, 2, K], F32, tag="gelTs")
        nc.scalar.copy(gelT, gelT_ps)
        enhT_ps = psum.tile([P, DC, K], F32, tag="enhT")
        for dci in range(DC):
            for hc in range(2):
                nc.tensor.matmul(enhT_ps[:, dci, :],
                                 lhsT=en_w2_s[:, hc, dci, :],
                                 rhs=gelT[:, hc, :],
                                 start=(hc == 0), stop=(hc == 1))
        enhT = singles.tile([P, DC, K], F32)
        nc.scalar.copy(enhT, enhT_ps)
        if general:
            for c in range(DC):
                nc.vector.tensor_scalar_add(enhT[:, c, :], enhT[:, c, :],
                                            en_b2t[:, c:c + 1])
                nc.vector.tensor_mul(enhT[:, c, :], enhT[:, c, :], valid_r)
        nc.sync.dma_start(enhanced_t, enhT)

        # ---- type MLP on enhanced
        hty_ps = psum.tile([K, H], F32, tag="hmm")
        for c in range(DC):
            nc.tensor.matmul(hty_ps, lhsT=enhT[:, c, :], rhs=ty_w1_s[:, c, :],
                             start=(c == 0), stop=(c == DC - 1))
        gel_ty = mlp_head(hty_ps, "ty")
        gtyT_ps = psum.tile([P, 2, K], F32, tag="gelT")
        for hc in range(2):
            nc.tensor.matmul(gtyT_ps[:, hc, :],
                             lhsT=gel_ty[:, hc * P:(hc + 1) * P], rhs=eye_s,
                             start=True, stop=True)
        gtyT = work.tile([P, 2, K], F32, tag="gtyTs")
        nc.scalar.copy(gtyT, gtyT_ps)
        log_ps = psum.tile([K, T], F32, tag="log")
        for hc in range(2):
            nc.tensor.matmul(log_ps, lhsT=gtyT[:, hc, :],
                             rhs=ty_w2_s[:, hc, :],
                             start=(hc == 0), stop=(hc == 1))
        logit = work.tile([K, T], F32, tag="logit")
        nc.scalar.copy(logit, log_ps)
        if general:
            nc.vector.tensor_add(logit, logit, gen_s["ty_b2"])
        nc.sync.dma_start(logits_o, logit)

        # softmax along T
        mx = work.tile([K, 1], F32, tag="mx")
        nc.vector.tensor_reduce(mx, logit, axis=mybir.AxisListType.X,
                                op=OP.max)
        sh = work.tile([K, T], F32, tag="sh")
        nc.vector.tensor_scalar(sh, logit, scalar1=mx, scalar2=None,
                                op0=OP.subtract)
        ex = work.tile([K, T], F32, tag="ex")
        nc.scalar.activation(ex, sh, AF.Exp)
        sm = work.tile([K, 1], F32, tag="sm")
        nc.vector.tensor_reduce(sm, ex, axis=mybir.AxisListType.X, op=OP.add)
        rs = work.tile([K, 1], F32, tag="rs")
        nc.vector.reciprocal(rs, sm)
        pr = work.tile([K, T], F32, tag="pr")
        nc.vector.tensor_scalar_mul(pr, ex, rs)
        nc.sync.dma_start(probs_o, pr)

        esc = work.tile([K, 1], F32, tag="esc")
        nc.vector.tensor_mul(esc, topv_s, envl_s)
        nc.sync.dma_start(escore_o, esc)

    nc.compile()
    return nc


def _get(key, builder):
    if key not in _cache:
        _cache[key] = builder()
    return _cache[key]


def _sigmoid64(x):
    return (1.0 / (1.0 + np.exp(-x.astype(np.float64)))).astype(np.float32)


def _rle(sc, mask_row):
    """Run-length encode spans; mirrors the reference's segment math."""
    ab = (sc >= THRESH) & (mask_row > 0)
    prev = np.zeros_like(ab)
    prev[1:] = ab[:-1]
    start = ab & ~prev
    sid = np.maximum(np.cumsum(start.astype(np.int64)) - 1, 0)
    span_len = np.zeros(S, np.float32)
    span_sum = np.zeros(S, np.float32)
    np.add.at(span_len, sid[ab], np.float32(1.0))
    np.add.at(span_sum, sid[ab], sc[ab].astype(np.float32))
    denom = np.maximum(span_len, np.float32(1.0))
    avg = span_sum / denom
    valid = (span_len > 0) & (avg >= THRESH)
    sel = np.where(valid, avg, np.float32(-1.0))
    starts_idx = np.nonzero(start)[0]
    return ab, sid, span_len, denom, sel, starts_idx


def kernel(hidden_states, attention_mask,
           sc_w1, sc_b1, sc_g, sc_be, sc_w2, sc_b2,
           en_w1, en_b1, en_g, en_be, en_w2, en_b2,
           ty_w1, ty_b1, ty_g, ty_be, ty_w2, ty_b2):
    hidden_states = np.asarray(hidden_states, dtype=np.float32)
    attention_mask = np.asarray(attention_mask)
    ws = {k: np.asarray(v, dtype=np.float32) for k, v in dict(
        sc_w1=sc_w1, sc_b1=sc_b1, sc_g=sc_g, sc_be=sc_be, sc_w2=sc_w2,
        sc_b2=sc_b2, en_w1=en_w1, en_b1=en_b1, en_g=en_g, en_be=en_be,
        en_w2=en_w2, en_b2=en_b2, ty_w1=ty_w1, ty_b1=ty_b1, ty_g=ty_g,
        ty_be=ty_be, ty_w2=ty_w2, ty_b2=ty_b2).items()}

    general = not (
        np.all(ws["sc_b1"] == 0) and np.all(ws["sc_g"] == 1)
        and np.all(ws["sc_be"] == 0) and np.all(ws["en_b1"] == 0)
        and np.all(ws["en_g"] == 1) and np.all(ws["en_be"] == 0)
        and np.all(ws["en_b2"] == 0) and np.all(ws["ty_b1"] == 0)
        and np.all(ws["ty_g"] == 1) and np.all(ws["ty_be"] == 0)
        and np.all(ws["ty_b2"] == 0))

    mask_f = attention_mask.astype(np.float32)
    cores = list(range(NCORES))

    # ---- phase 1: per-token scores ------------------------------------
    # center w1 rows (in f64) so LN's mean is zero by construction
    w1c = (ws["sc_w1"].astype(np.float64)
           - ws["sc_w1"].astype(np.float64).mean(axis=1, keepdims=True)
           ).astype(np.float32)
    if general:
        nc1 = _get(("scorer", "f32g", NSUB), lambda: _build_scorer(
            True, F32, NSUB, 4))
        xt_cast = lambda a: np.ascontiguousarray(a.T)
        w1_cast = ws["sc_w1"]
    else:
        nc1 = _get(("scorer", "bf16", NSUB), lambda: _build_scorer(
            False, BF16, NSUB, 16))
        xt_cast = lambda a: np.ascontiguousarray(a.T).astype(
            ml_dtypes.bfloat16)
        w1_cast = w1c.astype(ml_dtypes.bfloat16)

    in_maps1 = []
    for r in range(B):
        m = {
            "xt": xt_cast(hidden_states[r]),
            "w1": w1_cast,
            "w2": np.ascontiguousarray(ws["sc_w2"].reshape(1, H)),
            "b2": ws["sc_b2"].reshape(1, 1),
            "maskt": np.ascontiguousarray(mask_f[r].reshape(NSUB, P).T),
        }
        if general:
            m["b1"] = ws["sc_b1"].reshape(1, H)
            m["g"] = ws["sc_g"].reshape(1, H)
            m["be"] = ws["sc_be"].reshape(1, H)
        in_maps1.append(m)
    r1 = bass_utils.run_bass_kernel_spmd(nc1, in_maps1, core_ids=cores)
    pre = np.stack([r1.results[r]["pre_out"].T.reshape(S) for r in range(B)])
    scores_dev = np.stack(
        [r1.results[r]["scores_out"].T.reshape(S) for r in range(B)])

    # ---- rescue: exact fp32 re-score of decision-critical tokens -------
    if not general:
        s_b = _sigmoid64(pre) * mask_f
        rescue_idx = []
        for r in range(B):
            ab, sid, span_len, denom, sel, starts_idx = _rle(s_b[r], mask_f[r])
            keep = set()
            band = np.nonzero(
                (np.abs(s_b[r] - THRESH) < BAND) & (mask_f[r] > 0))[0]
            nspans = len(starts_idx)

            def add_span(j):
                if 0 <= j < nspans:
                    st0 = int(starts_idx[j])
                    keep.update(range(st0, st0 + int(span_len[j])))

            for t in band:
                keep.add(int(t))
                for tt in (t - 1, t + 1):
                    if 0 <= tt < S and ab[tt]:
                        add_span(int(sid[tt]))
            for j in np.argsort(-sel, kind="stable")[:NCAND]:
                add_span(int(j))
            rescue_idx.append(np.array(sorted(keep), dtype=np.int64))

        ncre = _get(("scorer", "f32", RSUB), lambda: _build_scorer(
            False, F32, RSUB, RSUB))
        nchunks = max(1, max((len(i) + NRES - 1) // NRES for i in rescue_idx))
        for ch in range(nchunks):
            in_mapsr = []
            for r in range(B):
                idx = rescue_idx[r][ch * NRES:(ch + 1) * NRES]
                xr = np.zeros((NRES, D), np.float32)
                xr[:len(idx)] = hidden_states[r, idx]
                in_mapsr.append({
                    "xt": np.ascontiguousarray(xr.T),
                    "w1": w1c,
                    "w2": np.ascontiguousarray(ws["sc_w2"].reshape(1, H)),
                    "b2": ws["sc_b2"].reshape(1, 1),
                    "maskt": np.ones((P, RSUB), np.float32),
                })
            rr = bass_utils.run_bass_kernel_spmd(ncre, in_mapsr,
                                                 core_ids=cores)
            for r in range(B):
                idx = rescue_idx[r][ch * NRES:(ch + 1) * NRES]
                if len(idx) == 0:
                    continue
                pre_r = rr.results[r]["pre_out"].T.reshape(NRES)[:len(idx)]
                s_r = rr.results[r]["scores_out"].T.reshape(NRES)[:len(idx)]
                pre[r, idx] = pre_r
                scores_dev[r, idx] = s_r * mask_f[r, idx]

    # ---- host: RLE + top-k selection (index bookkeeping) ---------------
    s_sel = _sigmoid64(pre) * mask_f
    in_maps2 = []
    sel_meta = []
    for r in range(B):
        ab, sid, span_len, denom, sel, starts_idx = _rle(s_sel[r], mask_f[r])
        order = np.argsort(-sel, kind="stable")[:K]
        top_vals = sel[order]
        valid_k = (top_vals >= THRESH).astype(np.float32)
        lens_k = denom[order]
        nspans = len(starts_idx)

        xsel = np.zeros((NSELP, D), np.float32)
        ind = np.zeros((NSELP, K), np.float32)
        for k in range(K):
            j = int(order[k])
            if valid_k[k] == 0 or j >= nspans:
                continue
            st_j = int(starts_idx[j])
            ln = int(span_len[j])
            row0 = k * LMAX
            scale = np.float32(valid_k[k] / lens_k[k])
            if ln <= LMAX:
                xsel[row0:row0 + ln] = hidden_states[r, st_j:st_j + ln]
                ind[row0:row0 + ln, k] = scale
            else:
                nfit = LMAX - 1
                xsel[row0:row0 + nfit] = hidden_states[r, st_j:st_j + nfit]
                xsel[row0 + nfit] = np.add.reduce(
                    hidden_states[r, st_j + nfit:st_j + ln], axis=0,
                    dtype=np.float32)
                ind[row0:row0 + LMAX, k] = scale
        m = {
            "xsel": xsel,
            "ind": ind,
            "diagv": np.diag(valid_k).astype(np.float32),
            "eye": np.eye(K, dtype=np.float32),
            "envl": valid_k.reshape(K, 1),
            "topv": top_vals.reshape(K, 1).astype(np.float32),
            "en_w1": ws["en_w1"], "en_w2": ws["en_w2"],
            "ty_w1": ws["ty_w1"], "ty_w2": ws["ty_w2"],
        }
        if general:
            m.update({
                "en_b1": ws["en_b1"].reshape(1, H),
                "en_g": ws["en_g"].reshape(1, H),
                "en_be": ws["en_be"].reshape(1, H),
                "en_b2": ws["en_b2"].reshape(1, D),
                "ty_b1": ws["ty_b1"].reshape(1, H),
                "ty_g": ws["ty_g"].reshape(1, H),
                "ty_be": ws["ty_be"].reshape(1, H),
                "ty_b2": ws["ty_b2"].reshape(1, T),
            })
        in_maps2.append(m)
        sel_meta.append((top_vals, valid_k))

    nc2 = _get(("phase2", general), lambda: _build_phase2(general))
    r2 = bass_utils.run_bass_kernel_spmd(nc2, in_maps2, core_ids=cores)

    enhanced = np.stack([
        r2.results[r]["enhanced_t"].transpose(2, 1, 0).reshape(K, D)
        for r in range(B)])
    type_logits = np.stack([r2.results[r]["logits"] for r in range(B)])
    type_probs = np.stack([r2.results[r]["probs"] for r in range(B)])
    entity_scores = np.stack(
        [r2.results[r]["escore"].reshape(K) for r in range(B)])
    entity_types = np.argmax(type_logits, axis=-1).astype(np.int32)
    valid_k_out = np.stack([sel_meta[r][1] for r in range(B)])

    return (scores_dev.astype(np.float32), enhanced, type_logits,
            entity_scores, type_probs, entity_types, valid_k_out)
